# revision 1
# baseline (speedup 1.0000x reference)
"""Trainium2 Bass kernel for nn_ColumnUniform (GNN message passing).

Computes, for a graph with N nodes and E edges (edge_index = [row; col]):
    rowsum[n] = sum of edge_attr over edges with row index n
    out[e]    = edge_attr[e] / rowsum[col[e]]

Strategy (8 NeuronCores, SPMD, fully streaming device kernel):
  Sharding: node range c = [c*N/8, (c+1)*N/8). Core c receives
    - phase A: all edges whose ROW is in range c (for the row sums), and
    - phase B: all edges whose COL is in range c (for the scaling),
  so the per-range reciprocal table is produced and consumed on the same
  core: no inter-core communication at all.

  Layout (host, pure index manipulation): nodes of a range are grouped
  into (col-degree-class, row-degree-class) cells; each node gets a
  fixed-size slot of D cells in the phase-A stream (its row-edges,
  zero-padded) and of Ec cells in the phase-B stream (its col-edges).
  Degree classes are chosen from the data by a small DP so padding is a
  few percent.

  Device: phase A = windowed tensor_reduce (window D) over the resident
  phase-A stream -> rowsum per slot -> reciprocal (+1 Newton step), all
  kept in SBUF. Phase B = one broadcast-multiply per tile: each slot's
  reciprocal value times its Ec col-edge attrs, streamed out. The output
  leaves the device in slot order; the host inverts the (pure
  permutation) layout when unsharding.
"""
import sys

for _p in ("/opt/trn_rl_repo", "/root/.axon_site/_ro/trn_rl_repo"):
    if _p not in sys.path:
        sys.path.append(_p)

import numpy as np

import concourse.bass as bass
import concourse.mybir as mybir
from concourse.bass_utils import run_bass_kernel_spmd

DT = mybir.dt.float32

N_CORES = 8
P = 128               # SBUF partitions
ACHUNK = 2048         # phase-A load chunk (free-dim columns)
BCHUNK = 4096         # phase-B tile width budget (free-dim columns)
NCLASS = 12           # degree classes per side


# ----------------------------------------------------------------------------
# Host-side layout (the sharding strategy): integer index work only.
# ----------------------------------------------------------------------------

def _pick_classes(deg, K):
    """Choose <=K class ceilings for positive degrees minimizing total slot
    cells (DP over quantile candidates)."""
    d = deg[deg > 0]
    dmax = int(d.max())
    cnt = np.bincount(d, minlength=dmax + 1)
    csum = np.concatenate([[0], np.cumsum(cnt)])
    cand = np.unique(np.concatenate([
        np.quantile(d, np.linspace(0, 1, K * 4)).astype(np.int64), [dmax]]))
    cand = cand[cand > 0]
    M = len(cand)
    INF = float("inf")
    dp = np.full((K + 1, M), INF)
    par = np.zeros((K + 1, M), np.int64)
    for j in range(M):
        dp[1][j] = csum[cand[j] + 1] * cand[j]
    for k in range(2, K + 1):
        for j in range(1, M):
            pj = np.arange(j)
            costs = dp[k - 1][pj] + (csum[cand[j] + 1] - csum[cand[pj] + 1]) * cand[j]
            i = int(np.argmin(costs))
            dp[k][j] = costs[i]
            par[k][j] = pj[i]
    ks = int(np.argmin(dp[:, M - 1]))
    out = []
    k, j = ks, M - 1
    while k >= 1:
        out.append(int(cand[j]))
        j = int(par[k][j])
        k -= 1
    return np.array(sorted(out), np.int64)


def prepare(edge_index, edge_attr, n_nodes):
    row = np.asarray(edge_index[0]).astype(np.int64)
    col = np.asarray(edge_index[1]).astype(np.int64)
    attr = np.asarray(edge_attr, dtype=np.float32)
    E = attr.shape[0]
    N = int(n_nodes)
    NR = (N + N_CORES - 1) // N_CORES

    rd = np.bincount(row, minlength=N)
    cd = np.bincount(col, minlength=N)
    clD = _pick_classes(rd, NCLASS)
    clE = _pick_classes(cd, NCLASS)
    Dn = clD[np.searchsorted(clD, np.maximum(rd, 1))]
    En = np.where(cd > 0, clE[np.searchsorted(clE, np.maximum(cd, 1))], 0)

    core = np.minimum(np.arange(N) // NR, N_CORES - 1)
    # cell id = (E-class index + 1 [0 for cd==0], D-class index), E-major
    eidx = np.where(cd > 0, np.searchsorted(clE, np.maximum(cd, 1)) + 1, 0)
    didx = np.searchsorted(clD, np.maximum(rd, 1))
    cellkey = eidx * (len(clD) + 1) + didx

    # node order within (core, cell); nodes ascending keeps things stable
    order = np.lexsort((np.arange(N), cellkey, core))
    oc = core[order]
    ock = cellkey[order]
    # group start positions
    grp = oc * (cellkey.max() + 1) + ock
    starts = np.concatenate([[0], np.nonzero(np.diff(grp))[0] + 1])
    gstart = np.zeros(N, np.int64)
    gstart[starts] = starts
    np.maximum.accumulate(gstart, out=gstart)
    krank = np.arange(N) - gstart          # slot index k within (core, cell)
    kn = np.empty(N, np.int64)
    kn[order] = krank

    # per-cell max count over cores -> shared geometry
    ucell, uinv = np.unique(cellkey, return_inverse=True)
    CNC = len(ucell)
    counts = np.zeros((N_CORES, CNC), np.int64)
    np.add.at(counts, (core, uinv), 1)
    smax = counts.max(axis=0)
    ka = -(-smax // P)                      # column groups per cell
    cellD = clD[ucell % (len(clD) + 1)]
    cellE = np.where(ucell >= (len(clD) + 1),
                     clE[np.maximum(ucell // (len(clD) + 1) - 1, 0)], 0)

    wa_w = ka * cellD
    wb_w = ka * cellE                       # 0 for the cd==0 cells
    ca = np.concatenate([[0], np.cumsum(wa_w)])[:-1]
    cv = np.concatenate([[0], np.cumsum(ka)])[:-1]
    cb = np.concatenate([[0], np.cumsum(wb_w)])[:-1]
    WA = int(wa_w.sum())
    WV = int(ka.sum())
    WB = int(wb_w.sum())

    # per-node placement
    ci = uinv                               # cell index per node
    pn = kn % P
    jn = kn // P
    acol0 = ca[ci] + jn * cellD[ci]
    bcol0 = cb[ci] + jn * cellE[ci]

    # ranks of edges within row / col
    def ranks(keys):
        ptr = np.zeros(N + 1, np.int64)
        np.cumsum(np.bincount(keys, minlength=N), out=ptr[1:])
        prm = np.argsort(keys, kind="stable")
        r = np.arange(E, dtype=np.int64) - ptr[keys[prm]]
        out = np.empty(E, np.int64)
        out[prm] = r
        return out

    rrank = ranks(row)
    crank = ranks(col)

    # scatter attr into per-core A and B streams
    attr_a = np.zeros((N_CORES, P, WA), np.float32)
    attr_b = np.zeros((N_CORES, P, WB), np.float32)
    fa = core[row] * (P * WA) + pn[row] * WA + acol0[row] + rrank
    attr_a.reshape(-1)[fa] = attr
    fb = core[col] * (P * WB) + pn[col] * WB + bcol0[col] + crank
    attr_b.reshape(-1)[fb] = attr

    in_maps = [{"attr_a": attr_a[c], "attr_b": attr_b[c]} for c in range(N_CORES)]

    # cells in stream order for the device program
    cells = []
    for i in range(CNC):
        cells.append(dict(D=int(cellD[i]), E=int(cellE[i]), ka=int(ka[i]),
                          ca=int(ca[i]), cv=int(cv[i]), cb=int(cb[i])))
    geom = dict(WA=WA, WB=WB, WV=WV, cells=cells)
    # info to unshard: position of each edge in the B stream of its core
    fb_local = pn[col] * WB + bcol0[col] + crank
    return in_maps, geom, (core[col], fb_local)


def unshard(results, E, geom, binfo):
    bcore, fb_local = binfo
    outs = np.stack([results[c]["out"].reshape(-1) for c in range(N_CORES)])
    return outs[bcore, fb_local]


# ----------------------------------------------------------------------------
# Device program
# ----------------------------------------------------------------------------

def build_program(geom, debug=False):
    WA = geom["WA"]
    WB = geom["WB"]
    WV = geom["WV"]
    cells = geom["cells"]

    nc = bass.Bass()
    attr_a = nc.declare_dram_parameter("attr_a", [P, WA], DT, isOutput=False)
    attr_b = nc.declare_dram_parameter("attr_b", [P, WB], DT, isOutput=False)
    out_ext = nc.declare_dram_parameter("out", [P, WB], DT, isOutput=True)
    if debug:
        v_dbg = nc.declare_dram_parameter("v_dbg", [P, WV], DT, isOutput=True)
        rs_dbg = nc.declare_dram_parameter("rs_dbg", [P, WV], DT, isOutput=True)
        a_dbg = nc.declare_dram_parameter("a_dbg", [P, WA], DT, isOutput=True)

    # phase-A load chunks
    na = (WA + ACHUNK - 1) // ACHUNK
    # last chunk needed by each cell
    cell_chunk = [((c["ca"] + c["ka"] * c["D"] - 1) // ACHUNK) for c in cells]

    # phase-B groups: one load+store of <=BCHUNK contiguous columns, with
    # per-cell broadcast-multiply segments inside (split at slot boundaries)
    groups = []  # (g0, width, [(off, vcol0, k, e), ...])
    cur0 = None
    cur_w = 0
    cur_segs = []
    for c in cells:
        e = c["E"]
        if e == 0:
            continue
        k0 = 0
        while k0 < c["ka"]:
            if cur0 is None:
                cur0, cur_w, cur_segs = c["cb"] + k0 * e, 0, []
            room = (BCHUNK - cur_w) // e
            if room == 0:
                groups.append((cur0, cur_w, cur_segs))
                cur0, cur_w, cur_segs = c["cb"] + k0 * e, 0, []
                room = BCHUNK // e
            k = min(room, c["ka"] - k0)
            if (cur_segs and cur_segs[-1][3] == e
                    and cur_segs[-1][0] + cur_segs[-1][2] * e == cur_w
                    and cur_segs[-1][1] + cur_segs[-1][2] == c["cv"] + k0):
                off, v0, pk, _ = cur_segs[-1]
                cur_segs[-1] = (off, v0, pk + k, e)
            else:
                cur_segs.append((cur_w, c["cv"] + k0, k, e))
            cur_w += k * e
            k0 += k
    if cur0 is not None and cur_w:
        groups.append((cur0, cur_w, cur_segs))
    NG = len(groups)
    NBUF = 4

    from contextlib import ExitStack
    with ExitStack() as ctx:
        block = ctx.enter_context(nc.Block())
        sA = [ctx.enter_context(nc.semaphore(f"sA{i}")) for i in range(na)]
        sRed = ctx.enter_context(nc.semaphore("sRed"))   # cells reduced
        sV = ctx.enter_context(nc.semaphore("sV"))       # v table ready
        sBin = [ctx.enter_context(nc.semaphore(f"sBin{i}")) for i in range(NBUF)]
        sMul = ctx.enter_context(nc.semaphore("sMul"))   # B tiles multiplied
        sBout = [ctx.enter_context(nc.semaphore(f"sBout{i}")) for i in range(NBUF)]
        sDbg = ctx.enter_context(nc.semaphore("sDbg"))

        A_sb = ctx.enter_context(nc.sbuf_tensor("A_sb", [P, WA], DT))
        v_sb = ctx.enter_context(nc.sbuf_tensor("v_sb", [P, WV], DT))
        t_sb = ctx.enter_context(nc.sbuf_tensor("t_sb", [P, WV], DT))
        bt = [ctx.enter_context(nc.sbuf_tensor(f"bt{i}", [P, BCHUNK], DT))
              for i in range(NBUF)]

        @block.sync
        def _(sync):
            for i in range(na):
                w0 = i * ACHUNK
                w1 = min(WA, w0 + ACHUNK)
                sync.dma_start(out=A_sb[:, w0:w1], in_=attr_a[:, w0:w1]).then_inc(sA[i], 16)
            for g, (g0, w, segs) in enumerate(groups):
                if g >= NBUF:
                    # same buffer's previous store completed
                    sync.wait_ge(sBout[g % NBUF], 16 * ((g - NBUF) // NBUF + 1))
                sync.dma_start(out=bt[g % NBUF][:, :w], in_=attr_b[:, g0:g0 + w]).then_inc(sBin[g % NBUF], 16)

        @block.vector
        def _(vector):
            for i, c in enumerate(cells):
                c0 = c["ca"] // ACHUNK
                for ch in range(c0, cell_chunk[i] + 1):
                    vector.wait_ge(sA[ch], 16)
                ka, D, ca, cv = c["ka"], c["D"], c["ca"], c["cv"]
                src = A_sb[:, ca:ca + ka * D].rearrange("p (k d) -> p k d", d=D)
                vector.tensor_reduce(
                    out=v_sb[:, cv:cv + ka], in_=src,
                    axis=mybir.AxisListType.X, op=mybir.AluOpType.add,
                ).then_inc(sRed, 1)
            # v = 1/rowsum with one Newton refinement
            vector.wait_ge(sRed, len(cells))
            if debug:
                vector.wait_ge(sDbg, 16)
            vector.reciprocal(t_sb[:, :], v_sb[:, :])
            vector.tensor_mul(v_sb[:, :], v_sb[:, :], t_sb[:, :])      # x*r
            vector.tensor_scalar(out=v_sb[:, :], in0=v_sb[:, :],
                                 scalar1=-1.0, scalar2=2.0,
                                 op0=mybir.AluOpType.mult, op1=mybir.AluOpType.add)
            vector.tensor_mul(v_sb[:, :], v_sb[:, :], t_sb[:, :]).then_inc(sV, 1)
            # phase-B broadcast multiplies (in place on the loaded tile)
            vector.wait_ge(sV, 1)
            for g, (g0, w, segs) in enumerate(groups):
                vector.wait_ge(sBin[g % NBUF], 16 * (g // NBUF + 1))
                for si, (off, v0, k, e) in enumerate(segs):
                    dst = bt[g % NBUF][:, off:off + k * e].rearrange(
                        "p (k e) -> p k e", e=e)
                    inst = vector.tensor_tensor(
                        out=dst, in0=v_sb[:, v0:v0 + k, None].to_broadcast([P, k, e]),
                        in1=dst, op=mybir.AluOpType.mult,
                    )
                    if si == len(segs) - 1:
                        inst.then_inc(sMul, 1)

        @block.scalar
        def _(scalar):
            if debug:
                scalar.wait_ge(sRed, len(cells))
                scalar.dma_start(out=rs_dbg[:, :], in_=v_sb[:, :]).then_inc(sDbg, 16)
                scalar.wait_ge(sV, 1)
                scalar.dma_start(out=v_dbg[:, :], in_=v_sb[:, :]).then_inc(sDbg, 16)
                scalar.dma_start(out=a_dbg[:, :], in_=A_sb[:, :]).then_inc(sDbg, 16)
                scalar.wait_ge(sDbg, 48)
            for g, (g0, w, segs) in enumerate(groups):
                scalar.wait_ge(sMul, g + 1)
                scalar.dma_start(out=out_ext[:, g0:g0 + w], in_=bt[g % NBUF][:, :w]).then_inc(sBout[g % NBUF], 16)

    return nc


# ----------------------------------------------------------------------------
# Entry point
# ----------------------------------------------------------------------------

def kernel(edge_index, edge_attr, N):
    import os
    edge_index = np.asarray(edge_index)
    edge_attr = np.asarray(edge_attr)
    E = edge_attr.shape[0]
    in_maps, geom, binfo = prepare(edge_index, edge_attr, int(N))
    nc = build_program(geom, debug=os.environ.get("KDBG") not in (None, "", "0"))
    trace = os.environ.get("KTRACE") not in (None, "", "0")
    if trace:
        import types
        import antenv
        if "antenv.axon_hooks" not in sys.modules:
            mod = types.ModuleType("antenv.axon_hooks")
            _h = [None]
            mod.set_axon_ntff_profile_hook = lambda h: _h.__setitem__(0, h)
            mod.get_axon_ntff_profile_hook = lambda: _h[0]
            sys.modules["antenv.axon_hooks"] = mod
            antenv.axon_hooks = mod
            from trn_agent_boot.trn_boot import _ntff_profile_via_ctypes
            mod.set_axon_ntff_profile_hook(
                _ntff_profile_via_ctypes("/opt/axon/libaxon_pjrt.so"))
    res = run_bass_kernel_spmd(nc, in_maps, list(range(N_CORES)), trace=trace)
    kernel.last = (res, in_maps, geom)
    return unshard(res.results, E, geom, binfo)


if __name__ == "__main__":
    rng = np.random.default_rng(0)
    N = 4096
    E = 65536
    row = np.concatenate([np.arange(N, dtype=np.int32),
                          rng.integers(0, N, E - N, dtype=np.int32)])
    col = rng.integers(0, N, E, dtype=np.int32)
    attr = rng.random(E, dtype=np.float32) * 0.9 + 0.1
    out = kernel(np.stack([row, col]), attr, N)
    rowsum = np.zeros(N, np.float64)
    np.add.at(rowsum, row, attr.astype(np.float64))
    exp = (1.0 / rowsum)[col] * attr
    err = np.abs(out - exp) / np.abs(exp)
    print("max rel err:", err.max())



# revision 2
# speedup vs baseline: 2.0076x; 2.0076x over previous
"""Trainium2 Bass kernel for nn_ColumnUniform (GNN message passing), v2.

out[e] = edge_attr[e] / rowsum(edge_attr)[col[e]]   for 20M edges, 1M nodes.

Sharding: node range per core (8 cores). Core c gets the edges whose ROW is
in its range (A stream, for rowsums) and the edges whose COL is in its range
(B stream, for scaling); the reciprocal table is produced and consumed on
the same core, so there is no inter-core communication.

Layout (host, pure index manipulation; fp16 wire format):
  Nodes are classed by row degree (<=NCD DP-chosen ceilings D) and by col
  degree (<=NCE ceilings E); cell = (dclass, eclass), D-primary order.
  Per cell: K = ceil(max-core count/128) slot columns; node k -> partition
  k%128, column k//128. The v table [128, WV] has one slot per node.
    A stream: slot-major windows, edge i of row r at aoff + j*D + i.
      One windowed tensor_reduce per D-class (its cells are contiguous).
    B stream: plane-interleaved, edge i of col c at boff + i*K + j.
      One broadcast multiply per cell: [P, E, K] *= vh[:, v0:v0+K].
  Reciprocal + f32->f16 convert run on the scalar (Act) engine, off the
  DVE critical path. Stores stream out region-by-region behind the muls.
"""
import sys

for _p in ("/opt/trn_rl_repo", "/root/.axon_site/_ro/trn_rl_repo"):
    if _p not in sys.path:
        sys.path.append(_p)

import numpy as np

import concourse.bass as bass
import concourse.mybir as mybir
from concourse.bass_utils import run_bass_kernel_spmd

F32 = mybir.dt.float32
F16 = mybir.dt.float16

P = 128
N_CORES = 8
NCD = 8                # row-degree classes (A side)
NCE = 4                # col-degree classes (B side)
CHUNK = 4096           # load chunk width (f16 columns)
NREGION = 10           # output store regions


# ----------------------------------------------------------------------------
# Host-side layout: integer index work only.
# ----------------------------------------------------------------------------

def dp_classes(deg, K):
    deg = deg[deg > 0]
    dmax = int(deg.max())
    cnt = np.bincount(deg, minlength=dmax + 1).astype(np.int64)
    vals = np.nonzero(cnt)[0]
    vals = vals[vals > 0]
    csum = np.concatenate([[0], np.cumsum(cnt)])
    M = len(vals)
    INF = float("inf")
    dp = np.full((K + 1, M), INF)
    par = np.zeros((K + 1, M), np.int64)
    for j in range(M):
        dp[1][j] = csum[vals[j] + 1] * vals[j]
    for k in range(2, K + 1):
        for j in range(k - 1, M):
            costs = dp[k - 1][:j] + (csum[vals[j] + 1] - csum[vals[:j] + 1]) * vals[j]
            i = int(np.argmin(costs))
            dp[k][j] = costs[i]
            par[k][j] = i
    k = int(np.argmin(dp[:, M - 1]))
    out = []
    j = M - 1
    while k >= 1:
        out.append(int(vals[j]))
        j = int(par[k][j])
        k -= 1
    return np.array(sorted(out), np.int64)


def edge_ranks(keys, N, E):
    ptr = np.zeros(N + 1, np.int64)
    np.cumsum(np.bincount(keys, minlength=N), out=ptr[1:])
    prm = np.argsort(keys, kind="stable")
    r = np.arange(E, dtype=np.int64) - ptr[keys[prm]]
    out = np.empty(E, np.int64)
    out[prm] = r
    return out


def prepare(edge_index, edge_attr, n_nodes):
    row = np.asarray(edge_index[0]).astype(np.int64)
    col = np.asarray(edge_index[1]).astype(np.int64)
    attr16 = np.asarray(edge_attr, dtype=np.float32).astype(np.float16)
    E = row.shape[0]
    N = int(n_nodes)
    NR = (N + N_CORES - 1) // N_CORES

    rd = np.bincount(row, minlength=N)
    cd = np.bincount(col, minlength=N)
    clD = dp_classes(rd, NCD)
    clE = dp_classes(cd, NCE)
    ncd, nce = len(clD), len(clE)
    dcls = np.searchsorted(clD, np.maximum(rd, 1))
    ecls = np.searchsorted(clE, np.maximum(cd, 1))
    cell = dcls * nce + ecls
    NCELL = ncd * nce
    core = np.minimum(np.arange(N) // NR, N_CORES - 1)

    counts = np.zeros((N_CORES, NCELL), np.int64)
    np.add.at(counts, (core, cell), 1)
    order = np.lexsort((np.arange(N), cell, core))
    grp = core[order] * NCELL + cell[order]
    starts = np.concatenate([[0], np.nonzero(np.diff(grp))[0] + 1])
    gstart = np.zeros(N, np.int64)
    gstart[starts] = starts
    np.maximum.accumulate(gstart, out=gstart)
    kn = np.empty(N, np.int64)
    kn[order] = np.arange(N) - gstart

    K = -(-counts.max(axis=0) // P)
    Dc = clD[np.arange(NCELL) // nce]
    Ec = clE[np.arange(NCELL) % nce]
    cv = np.concatenate([[0], np.cumsum(K)])
    aoff = np.concatenate([[0], np.cumsum(K * Dc)])
    boff = np.concatenate([[0], np.cumsum(K * Ec)])
    WV, WA, WB = int(cv[-1]), int(aoff[-1]), int(boff[-1])

    pn = kn % P
    jn = kn // P

    rrank = edge_ranks(row, N, E)
    crank = edge_ranks(col, N, E)

    acol = aoff[cell[row]] + jn[row] * Dc[cell[row]] + rrank
    fa = core[row] * (P * WA) + pn[row] * WA + acol
    attr_a = np.zeros(N_CORES * P * WA, np.float16)
    attr_a[fa] = attr16
    attr_a = attr_a.reshape(N_CORES, P, WA)

    bcol = boff[cell[col]] + crank * K[cell[col]] + jn[col]
    fb = core[col] * (P * WB) + pn[col] * WB + bcol
    attr_b = np.zeros(N_CORES * P * WB, np.float16)
    attr_b[fb] = attr16
    attr_b = attr_b.reshape(N_CORES, P, WB)

    classes = []
    for d in range(ncd):
        c0, c1 = d * nce, (d + 1) * nce
        classes.append(dict(D=int(clD[d]), a0=int(aoff[c0]), a1=int(aoff[c1]),
                            v0=int(cv[c0]), v1=int(cv[c1])))
    cells = []
    for c in range(NCELL):
        if K[c] == 0 or Ec[c] == 0:
            continue
        cells.append(dict(E=int(Ec[c]), K=int(K[c]), b0=int(boff[c]),
                          v0=int(cv[c]), d=int(c // nce)))
    geom = dict(WA=WA, WB=WB, WV=WV, classes=classes, cells=cells)
    binfo = (core[col], pn[col] * WB + bcol)
    in_maps = [{"attr_a": attr_a[c], "attr_b": attr_b[c]}
               for c in range(N_CORES)]
    return in_maps, geom, binfo


def unshard(results, geom, binfo):
    bcore, fb_local = binfo
    outs = np.stack([np.asarray(results[c]["out"]).reshape(-1)
                     for c in range(N_CORES)])
    return outs[bcore, fb_local].astype(np.float32)


# ----------------------------------------------------------------------------
# Device program
# ----------------------------------------------------------------------------

def build_program(geom):
    WA, WB, WV = geom["WA"], geom["WB"], geom["WV"]
    classes = geom["classes"]
    cells = geom["cells"]

    nc = bass.Bass()
    attr_a = nc.declare_dram_parameter("attr_a", [P, WA], F16, isOutput=False)
    attr_b = nc.declare_dram_parameter("attr_b", [P, WB], F16, isOutput=False)
    out_ext = nc.declare_dram_parameter("out", [P, WB], F16, isOutput=True)

    achunks = [(w, min(WA, w + CHUNK)) for w in range(0, WA, CHUNK)]
    bchunks = [(w, min(WB, w + CHUNK)) for w in range(0, WB, CHUNK)]
    na = len(achunks)

    # A chunk index needed by each class; B chunk index needed by each cell
    def a_hi(cl):
        return (cl["a1"] - 1) // CHUNK

    def b_hi(ce):
        return (ce["b0"] + ce["E"] * ce["K"] - 1) // CHUNK

    # output store regions: ~NREGION splits at cell boundaries
    regions = []
    tgt = (WB + NREGION - 1) // NREGION
    r0, nmul = 0, 0
    for ci, ce in enumerate(cells):
        end = ce["b0"] + ce["E"] * ce["K"]
        nmul += 1
        if end - r0 >= tgt or ci == len(cells) - 1:
            regions.append((r0, end, nmul))
            r0 = end
    assert regions[-1][1] == WB

    from contextlib import ExitStack
    with ExitStack() as ctx:
        block = ctx.enter_context(nc.Block())
        sLoad = ctx.enter_context(nc.semaphore("sLoad"))
        sRed = ctx.enter_context(nc.semaphore("sRed"))
        sV = ctx.enter_context(nc.semaphore("sV"))
        sMul = ctx.enter_context(nc.semaphore("sMul"))
        sOut = ctx.enter_context(nc.semaphore("sOut"))

        A_sb = ctx.enter_context(nc.sbuf_tensor("A_sb", [P, WA], F16))
        B_sb = ctx.enter_context(nc.sbuf_tensor("B_sb", [P, WB], F16))
        rs = ctx.enter_context(nc.sbuf_tensor("rs", [P, WV], F32))
        v32 = ctx.enter_context(nc.sbuf_tensor("v32", [P, WV], F32))
        vh = ctx.enter_context(nc.sbuf_tensor("vh", [P, WV], F16))

        @block.sync
        def _(sync):
            for w0, w1 in achunks:
                sync.dma_start(out=A_sb[:, w0:w1],
                               in_=attr_a[:, w0:w1]).then_inc(sLoad, 16)
            for w0, w1 in bchunks:
                sync.dma_start(out=B_sb[:, w0:w1],
                               in_=attr_b[:, w0:w1]).then_inc(sLoad, 16)

        @block.vector
        def _(vector):
            for cl in classes:
                vector.wait_ge(sLoad, 16 * (a_hi(cl) + 1))
                D, a0, a1, v0, v1 = cl["D"], cl["a0"], cl["a1"], cl["v0"], cl["v1"]
                src = A_sb[:, a0:a1].rearrange("p (k d) -> p k d", d=D)
                vector.tensor_reduce(
                    out=rs[:, v0:v1], in_=src,
                    axis=mybir.AxisListType.X, op=mybir.AluOpType.add,
                ).then_inc(sRed, 1)
            for ce in cells:
                vector.wait_ge(sLoad, 16 * (na + b_hi(ce) + 1))
                vector.wait_ge(sV, ce["d"] + 1)
                E, K, b0, v0 = ce["E"], ce["K"], ce["b0"], ce["v0"]
                dst = B_sb[:, b0:b0 + E * K].rearrange("p (e k) -> p e k", k=K)
                vector.tensor_tensor(
                    out=dst, in0=vh[:, None, v0:v0 + K].to_broadcast([P, E, K]),
                    in1=dst, op=mybir.AluOpType.mult,
                ).then_inc(sMul, 1)

        @block.scalar
        def _(scalar):
            for d, cl in enumerate(classes):
                scalar.wait_ge(sRed, d + 1)
                v0, v1 = cl["v0"], cl["v1"]
                scalar.add_instruction(mybir.InstActivation(
                    name=nc.get_next_instruction_name(),
                    func=mybir.ActivationFunctionType.Reciprocal,
                    ins=[scalar.lower_ap(rs[:, v0:v1]),
                         mybir.ImmediateValue(dtype=F32, value=0.0),
                         mybir.ImmediateValue(dtype=F32, value=1.0),
                         mybir.ImmediateValue(dtype=F32, value=0.0)],
                    outs=[scalar.lower_ap(v32[:, v0:v1])]))
                scalar.activation(out=vh[:, v0:v1], in_=v32[:, v0:v1],
                                  func=mybir.ActivationFunctionType.Copy,
                                  ).then_inc(sV, 1)
            for r0, r1, nmul in regions:
                scalar.wait_ge(sMul, nmul)
                scalar.dma_start(out=out_ext[:, r0:r1],
                                 in_=B_sb[:, r0:r1]).then_inc(sOut, 16)

    return nc


# ----------------------------------------------------------------------------
# Entry point
# ----------------------------------------------------------------------------

def kernel(edge_index, edge_attr, N):
    import os
    in_maps, geom, binfo = prepare(edge_index, edge_attr, int(N))
    nc = build_program(geom)
    trace = os.environ.get("KTRACE") not in (None, "", "0")
    if trace:
        import types
        import antenv
        if "antenv.axon_hooks" not in sys.modules:
            mod = types.ModuleType("antenv.axon_hooks")
            _h = [None]
            mod.set_axon_ntff_profile_hook = lambda h: _h.__setitem__(0, h)
            mod.get_axon_ntff_profile_hook = lambda: _h[0]
            sys.modules["antenv.axon_hooks"] = mod
            antenv.axon_hooks = mod
            from trn_agent_boot.trn_boot import _ntff_profile_via_ctypes
            mod.set_axon_ntff_profile_hook(
                _ntff_profile_via_ctypes("/opt/axon/libaxon_pjrt.so"))
    res = run_bass_kernel_spmd(nc, in_maps, list(range(N_CORES)), trace=trace)
    kernel.last = (res, in_maps, geom)
    return unshard(res.results, geom, binfo)


if __name__ == "__main__":
    rng = np.random.default_rng(0)
    N = 4096
    E = 65536
    row = np.concatenate([np.arange(N, dtype=np.int32),
                          rng.integers(0, N, E - N, dtype=np.int32)])
    col = rng.integers(0, N, E, dtype=np.int32)
    attr = rng.random(E, dtype=np.float32) * 0.9 + 0.1
    out = kernel(np.stack([row, col]), attr, N)
    rowsum = np.zeros(N, np.float64)
    np.add.at(rowsum, row, attr.astype(np.float64))
    exp = (1.0 / rowsum)[col] * attr
    err = np.abs(out - exp) / np.abs(exp)
    print("max rel err:", err.max())
